# revision 2
# baseline (speedup 1.0000x reference)
"""Trainium2 Bass kernel for nn_Conv2d_20590073217670.

Conv2d: input [32,64,64,64] (NCHW), weight [576,128] (unfold layout:
row = ci*9 + a*3 + b for tap (a,b)), bias [1,128,1,1], stride 1, pad 1.
Output [32,128,64,64].

Strategy: data-parallel over batch — 4 images per NeuronCore, 8 cores.
Per image, implicit GEMM: out[co, y, x] = sum_{a,b,ci} W[ci,a,b,co] *
img[ci, y+a-1, x+b-1].  SBUF holds the image on partitions 0:64 and a
one-row-down shifted copy on partitions 64:128, so a single K=128
matmul accumulates two vertical taps (a, a+1) at once.  The rounded
fp32r image is stored column-padded ([128, 64, 66], zero borders), so
every matmul is a full 64-wide slide satisfying the fp32r ISA
restrictions (even innermost count, 8B-aligned full-bank PSUM output).
Row borders are handled by restricting output rows (PSUM has_written
zero-fill keeps partial accumulation exact).  DVE produces all matmul
inputs (fp32->fp32r rounding) and evicts PSUM with a fused bias add.
"""
import sys

for _p in ("/opt/trn_rl_repo", "/root/.axon_site/_ro/trn_rl_repo"):
    if _p not in sys.path:
        sys.path.append(_p)

import numpy as np
from contextlib import ExitStack

import concourse.bacc as bacc
import concourse.tile as tile
from concourse import mybir
from concourse.bass_utils import run_bass_kernel_spmd

f32 = mybir.dt.float32
f32r = mybir.dt.float32r

N_CORES = 8
NB = 4  # images per core


def build_nc():
    nc = bacc.Bacc()
    x = nc.declare_dram_parameter("x", [NB, 64, 64, 64], f32, isOutput=False)
    w = nc.declare_dram_parameter("w", [576, 128], f32, isOutput=False)
    bias = nc.declare_dram_parameter("b", [128, 1], f32, isOutput=False)
    out = nc.declare_dram_parameter("out", [NB, 128, 64, 64], f32, isOutput=True)

    with tile.TileContext(nc) as tc, ExitStack() as ctx:
        const = ctx.enter_context(tc.tile_pool(name="const", bufs=1))
        xs_pool = ctx.enter_context(tc.tile_pool(name="xs", bufs=3))
        xr_pool = ctx.enter_context(tc.tile_pool(name="xr", bufs=3))
        ob_pool = ctx.enter_context(tc.tile_pool(name="ob", bufs=2))
        ps_pool = ctx.enter_context(tc.tile_pool(name="ps", bufs=8, space="PSUM"))

        # ---- weights: one [128, 9, 128] tile; partition p<64 holds channel
        # p's taps 0..8, partition 64+ci holds channel ci's taps 3..8 at
        # slots 0..5 (tap axis pre-shifted by -3).  Then the lhsT view
        # wr[:, t, :] pairs taps (t, t+3) across the partition halves:
        #   t in 0..2  -> taps (0,b) & (1,b)
        #   t in 3..5  -> taps (1,b) & (2,b)
        w3 = w[:].rearrange("(c t) m -> c t m", t=9)
        ws = const.tile([128, 9, 128], f32)
        wr = const.tile([128, 9, 128], f32r)
        bt = const.tile([128, 1], f32)
        zc = const.tile([128, 64, 1], f32)
        nc.sync.dma_start(out=ws[0:64, :, :], in_=w3)
        nc.sync.dma_start(out=ws[64:128, 0:6, :], in_=w3[:, 3:9, :])
        nc.sync.dma_start(out=bt[:], in_=bias[:])
        nc.vector.memset(zc[:], 0.0)
        nc.vector.tensor_copy(wr[0:64, :, :], ws[0:64, :, :])
        nc.vector.tensor_copy(wr[64:128, 0:6, :], ws[64:128, 0:6, :])

        for n in range(NB):
            xs = xs_pool.tile([128, 64, 64], f32)
            xr = xr_pool.tile([128, 64, 66], f32r)
            # image rows on partitions 0:64; one-row-down copy on 64:128
            nc.sync.dma_start(out=xs[0:64, :, :], in_=x[n])
            nc.sync.dma_start(out=xs[64:128, 0:63, :], in_=xs[0:64, 1:64, :])
            # fp32 -> fp32r rounding (DVE) into the column-padded layout;
            # upper-half row 63 is never read.  Zero border columns.
            nc.vector.tensor_copy(xr[:, 0:63, 1:65], xs[:, 0:63, :])
            nc.vector.tensor_copy(xr[0:64, 63, 1:65], xs[0:64, 63, :])
            nc.vector.tensor_copy(xr[:, :, 0:1], zc[:])
            nc.vector.tensor_copy(xr[:, :, 65:66], zc[:])

            osb = ob_pool.tile([128, 64, 64], f32)
            for blk in range(8):
                y0 = blk * 8
                P = ps_pool.tile([128, 8, 64], f32)
                if blk == 0:
                    pair_t, pr0 = 3, 0      # taps (1,2), rhs rows y0..y0+7
                else:
                    pair_t, pr0 = 0, y0 - 1  # taps (0,1), rhs rows y0-1..y0+6
                # b=1 first: full [8,64] coverage zero-fills the whole bank
                for k, b in enumerate((1, 0, 2)):
                    nc.tensor.matmul(
                        P[:, 0:8, :],
                        wr[:, pair_t + b, :],
                        xr[:, pr0:pr0 + 8, b:b + 64],
                        start=(k == 0), stop=False,
                    )
                # remaining vertical tap as K=64 single on partitions 0:64
                for k, b in enumerate((1, 0, 2)):
                    last = k == 2
                    if blk == 0:
                        # tap (0,b): out rows 1..7 read img rows 0..6
                        nc.tensor.matmul(
                            P[:, 1:8, :], wr[0:64, b, :],
                            xr[0:64, 0:7, b:b + 64],
                            start=False, stop=last,
                        )
                    elif blk == 7:
                        # tap (2,b): out rows 56..62 read img rows 57..63
                        nc.tensor.matmul(
                            P[:, 0:7, :], wr[0:64, 6 + b, :],
                            xr[0:64, 57:64, b:b + 64],
                            start=False, stop=last,
                        )
                    else:
                        nc.tensor.matmul(
                            P[:, 0:8, :], wr[0:64, 6 + b, :],
                            xr[0:64, y0 + 1:y0 + 9, b:b + 64],
                            start=False, stop=last,
                        )
                nc.vector.tensor_scalar_add(osb[:, y0:y0 + 8, :], P[:, :, :], bt[:])

            nc.sync.dma_start(out=out[n], in_=osb[:])

    nc.finalize()
    return nc


_NC = None


def _get_nc():
    global _NC
    if _NC is None:
        _NC = build_nc()
    return _NC


def kernel(**inputs) -> np.ndarray:
    x = np.ascontiguousarray(np.asarray(inputs["input"], dtype=np.float32))
    w = np.ascontiguousarray(np.asarray(inputs["weight"], dtype=np.float32))
    b = np.ascontiguousarray(
        np.asarray(inputs["bias"], dtype=np.float32).reshape(128, 1))
    nc = _get_nc()
    in_maps = [
        {"x": x[c * NB:(c + 1) * NB], "w": w, "b": b} for c in range(N_CORES)
    ]
    res = run_bass_kernel_spmd(nc, in_maps, list(range(N_CORES)))
    return np.concatenate([r["out"] for r in res.results], axis=0)
